# revision 23
# baseline (speedup 1.0000x reference)
"""Trainium2 Bass kernel for nn_MultiHeadAttention_67250597920960.

GQA attention block: q/k/v/gate projections, QK RMS-norm, RoPE, non-causal
SDPA, sigmoid gate, output projection.

Sharding: 8 cores = (batch b in {0,1}) x (kv-head group g in {0..3}).
Each core handles one batch element and one kv head (= 4 q heads):
  - slices wq/w_gate cols [g*512:(g+1)*512], wk/wv cols [g*128:(g+1)*128],
    w_proj rows [g*512:(g+1)*512]
  - produces a PARTIAL output [T, C]; host sums the 4 group partials per batch.

On-chip dataflow per core (all matmuls fp32 data via float32r PE mode, or bf16):
  Phase A (per 256-token chunk): DMA x rows -> PE-transpose to xT [c,t] ->
    qkv projections (out [t,768] psum) -> RMS-norm (ACT square, DVE reduce,
    sqrt/recip) -> RoPE (DVE, host-precomputed cos/sin tables with norm
    weights folded in) -> PE-transpose q,k to qT/kT [d,t]; v kept [t,d];
    gate projection emitted transposed [dout,t] -> DRAM roundtrip.
  Phase B (per q-head h, 512-token chunk): scores_T[s,t] = kT.T @ qT (PE),
    exp on ACT (scale=1/sqrt(D), no max-sub needed: |scores|<=sqrt(D)),
    yT[d,t] += v.T @ expT (PE), colsum[1,t] += ones.T @ expT (PE),
    recip colsum (DVE) -> DMA-broadcast over partitions ->
    ygT = yT * sigmoid(gateT) * recip (ACT+DVE).
  Phase C: out[t,e] = sum_h ygT_h.T @ w_proj_h (PE) -> DMA out.
"""

import math
import numpy as np

# ---- problem constants (hardcoded per spec) ----
B, T, C = 2, 2048, 2048
NH, NKV, D = 16, 4, 128
HG = NH // NKV          # q heads per core = 4
GD = HG * D             # 512
P = 128
TT_N = T // P           # 16 token tiles
CT_N = C // P           # 16 channel tiles
N_CORES = 8
RMS_EPS = 1e-6
SCALE = 1.0 / math.sqrt(D)

TCH = 256               # phase A token chunk
NCH = T // TCH          # 8 chunks
TC2 = 512               # phase B token chunk
NC2 = T // TC2          # 4 chunks

# "f32r": fp32 data, float32r matmul mode. "bf16": everything bf16.
DT_MODE = "f32r"


def _build_nc(dt_mode):
    import concourse.bass as bass  # noqa: F401
    import concourse.bacc as bacc
    import concourse.mybir as mybir
    import concourse.tile as tile
    from concourse.masks import make_identity

    fp32 = mybir.dt.float32
    if dt_mode == "bf16":
        DT_IN = mybir.dt.bfloat16   # dtype of x + transpose path
        DT_MM = mybir.dt.bfloat16   # dtype of matmul operands
    else:
        # float32r = TF32 PE mode: full-rate matmuls on fp32 data.
        # Every tensor consumed by an fp32r matmul must be produced with
        # dtype float32r (the BIR verifier's "rounded to FP32r" rule).
        DT_IN = mybir.dt.float32
        DT_MM = mybir.dt.float32r

    AF = mybir.ActivationFunctionType

    # Bacc (not raw Bass): its compile() passes split multi-sem waits into
    # EventSemaphore instructions (TPB insts allow at most 1 wait each).
    nc = bacc.Bacc("TRN2", target_bir_lowering=False, debug=False,
                   enable_asserts=False)

    x_d = nc.dram_tensor("x", [T, C], DT_MM, kind="ExternalInput").ap()
    ident_d = nc.dram_tensor("ident", [P, P], DT_MM, kind="ExternalInput").ap()
    wqkv_d = nc.dram_tensor("wqkv", [C, GD + 2 * D], DT_MM,
                            kind="ExternalInput").ap()
    wgate_d = nc.dram_tensor("wgate", [C, GD], DT_MM, kind="ExternalInput").ap()
    wproj_d = nc.dram_tensor("wproj", [GD, C], DT_MM, kind="ExternalInput").ap()
    ropeq_d = nc.dram_tensor("ropeq", [T, 256], fp32, kind="ExternalInput").ap()
    ropek_d = nc.dram_tensor("ropek", [T, 256], fp32, kind="ExternalInput").ap()
    out_d = nc.dram_tensor("out", [T, C], fp32, kind="ExternalOutput").ap()
    gate_dram = nc.dram_tensor("gatebuf", [GD, T], DT_IN).ap()
    rc_dram = nc.dram_tensor("rcbuf", [NC2 * HG, TC2], fp32).ap()

    with tile.TileContext(nc) as tc:
        with tc.tile_pool(name="persist", bufs=1) as persist:
            ident = persist.tile([P, P], DT_MM, tag="ident")
            nc.sync.dma_start(out=ident, in_=ident_d)
            ones_f = persist.tile([P, 1], fp32, tag="ones_f")
            nc.vector.memset(ones_f, 1.0)
            ones = persist.tile([P, 1], DT_MM, tag="ones")
            nc.scalar.copy(ones, ones_f)
            eps_t = persist.tile([P, 1], fp32, tag="eps")
            nc.vector.memset(eps_t, RMS_EPS)
            qT_sb = persist.tile([P, HG, T], DT_MM, tag="qT")
            kT_sb = persist.tile([P, T], DT_MM, tag="kT")
            v_sb = persist.tile([P, TT_N, P], DT_MM, tag="v")

            # ---------------- Phase A ----------------
            with tc.tile_pool(name="wA", bufs=1) as wA, \
                 tc.tile_pool(name="xT", bufs=1) as xTp, \
                 tc.tile_pool(name="xnat", bufs=2) as xnatp, \
                 tc.tile_pool(name="rope", bufs=4) as ropep, \
                 tc.tile_pool(name="scrA", bufs=2) as scrA, \
                 tc.tile_pool(name="gst", bufs=2) as gstp, \
                 tc.tile_pool(name="psA", bufs=2, space="PSUM") as psA, \
                 tc.tile_pool(name="psQKV", bufs=2, space="PSUM") as psQKV:

                # per-c-tile weight loads: keeps the per-matmul wait count low
                # (one fanned-out 6MB DMA -> too many sync waits on consumers)
                wqkv_sb = wA.tile([P, CT_N, GD + 2 * D], DT_MM, tag="wqkv")
                for ct in range(CT_N):
                    nc.gpsimd.dma_start(
                        out=wqkv_sb[:, ct, :],
                        in_=wqkv_d[ct * P:(ct + 1) * P, :])
                wgate_sb = wA.tile([P, CT_N, GD], DT_MM, tag="wgate")

                xT_sb = xTp.tile([P, CT_N, TCH], DT_MM, tag="xT")

                for ch in range(NCH):
                    # -- transpose x rows of this chunk into xT [c, t] --
                    for ti in range(TCH // P):
                        tt = ch * (TCH // P) + ti
                        xnat = xnatp.tile([P, C], DT_MM, tag="xnat")
                        nc.sync.dma_start(
                            out=xnat, in_=x_d[tt * P:(tt + 1) * P, :])
                        for cg in range(CT_N // 8):
                            tp_ps = psA.tile([P, 1024], DT_MM, tag="tp")
                            for j in range(8):
                                ct = cg * 8 + j
                                nc.tensor.transpose(
                                    tp_ps[:, j * P:(j + 1) * P],
                                    xnat[:, ct * P:(ct + 1) * P], ident)
                            nc.scalar.copy(
                                out=xT_sb[:, cg * 8:(cg + 1) * 8,
                                          ti * P:(ti + 1) * P],
                                in_=tp_ps.rearrange("p (j t) -> p j t", t=P))

                    if ch == 0:
                        for ct in range(CT_N):
                            nc.sync.dma_start(
                                out=wgate_sb[:, ct, :],
                                in_=wgate_d[ct * P:(ct + 1) * P, :])

                    # -- qkv projections + norm + rope per token tile --
                    qr_tiles = [None] * (TCH // P)
                    for ti in range(TCH // P):
                        tt = ch * (TCH // P) + ti
                        qkv_ps = psQKV.tile([P, GD + 2 * D], fp32, tag="qkv")
                        for ct in range(CT_N):
                            nc.tensor.matmul(
                                qkv_ps[:, 0:512],
                                (xT_sb[:, ct, ti * P:(ti + 1) * P]),
                                (wqkv_sb[:, ct, 0:512]),
                                start=(ct == 0), stop=(ct == CT_N - 1))
                        for ct in range(CT_N):
                            nc.tensor.matmul(
                                qkv_ps[:, 512:768],
                                (xT_sb[:, ct, ti * P:(ti + 1) * P]),
                                (wqkv_sb[:, ct, 512:768]),
                                start=(ct == 0), stop=(ct == CT_N - 1))

                        # RMS norm over d for q (4 heads) and k
                        sq = scrA.tile([P, 640], fp32, tag="sq")
                        nc.scalar.activation(sq, qkv_ps[:, 0:640], AF.Square)
                        ssum = scrA.tile([P, 5], fp32, tag="ssum")
                        nc.vector.reduce_sum(
                            ssum, sq.rearrange("p (h d) -> p h d", d=D),
                            axis=mybir.AxisListType.X)
                        rstd = scrA.tile([P, 5], fp32, tag="rstd")
                        nc.scalar.activation(rstd, ssum, AF.Sqrt,
                                             bias=eps_t, scale=1.0 / D)
                        nc.vector.reciprocal(rstd, rstd)
                        qn = scrA.tile([P, 640], fp32, tag="qn")
                        for hh in range(5):
                            nc.vector.tensor_scalar_mul(
                                qn[:, hh * D:(hh + 1) * D],
                                qkv_ps[:, hh * D:(hh + 1) * D],
                                rstd[:, hh:hh + 1])
                        # v: straight copy out of psum
                        nc.scalar.copy(out=v_sb[:, tt, :], in_=qkv_ps[:, 640:768])

                        # RoPE; tables already include q/k norm weights
                        rq = ropep.tile([P, 256], fp32, tag="rq")
                        nc.sync.dma_start(out=rq,
                                          in_=ropeq_d[tt * P:(tt + 1) * P, :])
                        rk = ropep.tile([P, 256], fp32, tag="rk")
                        nc.sync.dma_start(out=rk,
                                          in_=ropek_d[tt * P:(tt + 1) * P, :])
                        qr = scrA.tile([P, 640], DT_MM, tag="qr")
                        s1 = scrA.tile([P, HG, 64], fp32, tag="s1")
                        s2 = scrA.tile([P, HG, 64], fp32, tag="s2")
                        qn3 = qn[:, 0:512].rearrange("p (h d) -> p h d", d=D)
                        qr3 = qr[:, 0:512].rearrange("p (h d) -> p h d", d=D)

                        def bcast4(ap):
                            return ap.unsqueeze(1).to_broadcast((P, HG, 64))

                        # y1 = x1*A - x2*B ; y2 = x1*Csin + x2*Dcos
                        nc.vector.tensor_mul(s1, qn3[:, :, 0:64],
                                             bcast4(rq[:, 0:64]))
                        nc.vector.tensor_mul(s2, qn3[:, :, 64:128],
                                             bcast4(rq[:, 64:128]))
                        nc.vector.tensor_sub(qr3[:, :, 0:64], s1, s2)
                        nc.vector.tensor_mul(s1, qn3[:, :, 0:64],
                                             bcast4(rq[:, 128:192]))
                        nc.vector.tensor_mul(s2, qn3[:, :, 64:128],
                                             bcast4(rq[:, 192:256]))
                        nc.vector.tensor_add(qr3[:, :, 64:128], s1, s2)
                        # k rope
                        nc.vector.tensor_mul(s1[:, 0, :], qn[:, 512:576],
                                             rk[:, 0:64])
                        nc.vector.tensor_mul(s2[:, 0, :], qn[:, 576:640],
                                             rk[:, 64:128])
                        nc.vector.tensor_sub(qr[:, 512:576], s1[:, 0, :],
                                             s2[:, 0, :])
                        nc.vector.tensor_mul(s1[:, 0, :], qn[:, 512:576],
                                             rk[:, 128:192])
                        nc.vector.tensor_mul(s2[:, 0, :], qn[:, 576:640],
                                             rk[:, 192:256])
                        nc.vector.tensor_add(qr[:, 576:640], s1[:, 0, :],
                                             s2[:, 0, :])
                        qr_tiles[ti] = qr

                    # -- gate projection for this chunk, transposed output --
                    for j in range(HG):
                        g_ps = psA.tile([P, TCH], fp32, tag="tp")
                        for ct in range(CT_N):
                            nc.tensor.matmul(
                                g_ps,
                                (wgate_sb[:, ct, j * P:(j + 1) * P]),
                                (xT_sb[:, ct, :]),
                                start=(ct == 0), stop=(ct == CT_N - 1))
                        gs = gstp.tile([P, TCH], DT_IN, tag="gs")
                        nc.scalar.activation(gs, g_ps, AF.Sigmoid)
                        nc.sync.dma_start(
                            out=gate_dram[j * P:(j + 1) * P,
                                          ch * TCH:(ch + 1) * TCH],
                            in_=gs)

                    for ti in range(TCH // P):
                        tt = ch * (TCH // P) + ti
                        qr = qr_tiles[ti]
                        # transpose q heads and k into qT/kT [d, t]
                        tq_ps = psA.tile([P, 1024], DT_MM, tag="tp")
                        for h in range(HG + 1):
                            nc.tensor.transpose(
                                tq_ps[:, h * P:(h + 1) * P],
                                qr[:, h * P:(h + 1) * P], ident)
                        nc.scalar.copy(
                            out=qT_sb[:, :, tt * P:(tt + 1) * P],
                            in_=tq_ps[:, 0:512].rearrange(
                                "p (h t) -> p h t", t=P))
                        nc.scalar.copy(out=kT_sb[:, tt * P:(tt + 1) * P],
                                       in_=tq_ps[:, 512:640])

            # ---------------- Phase B ----------------
            with tc.tile_pool(name="ygT", bufs=1) as ygTp, \
                 tc.tile_pool(name="wC", bufs=1) as wC:
                ygT_sb = ygTp.tile([P, HG, T], DT_MM, tag="ygT")
                wproj_sb = wC.tile([P, HG, C], DT_MM, tag="wproj")
                for hd in range(HG):
                    nc.gpsimd.dma_start(
                        out=wproj_sb[:, hd, :],
                        in_=wproj_d[hd * P:(hd + 1) * P, :])

                with tc.tile_pool(name="expB", bufs=3) as expB, \
                     tc.tile_pool(name="gB", bufs=2) as gB, \
                     tc.tile_pool(name="smB", bufs=2) as smB, \
                     tc.tile_pool(name="psSC", bufs=2, space="PSUM") as psSC, \
                     tc.tile_pool(name="psY", bufs=3, space="PSUM") as psY, \
                     tc.tile_pool(name="psCS", bufs=1, space="PSUM") as psCS:

                    for c2 in range(NC2):
                        tsl = slice(c2 * TC2, (c2 + 1) * TC2)
                        for h in range(HG):
                            yT_ps = psY.tile([P, TC2], fp32, tag="yT")
                            cs_ps = psCS.tile([1, TC2], fp32, tag="cs")

                            def sc_pair(stp):
                                sc_ps = psSC.tile([P, 2 * TC2], fp32,
                                                  tag="sc")
                                for k in range(2):
                                    nc.tensor.matmul(
                                        sc_ps[:, k * TC2:(k + 1) * TC2],
                                        kT_sb[:, (2 * stp + k) * P:
                                              (2 * stp + k + 1) * P],
                                        qT_sb[:, h, tsl],
                                        start=True, stop=True)
                                e_sb = expB.tile([P, 2 * TC2], DT_MM,
                                                 tag="exp")
                                nc.scalar.activation(e_sb, sc_ps, AF.Exp,
                                                     scale=SCALE)
                                return e_sb

                            def yc_pair(stp, e_sb):
                                first, last = stp == 0, stp == TT_N // 2 - 1
                                for k in range(2):
                                    nc.tensor.matmul(
                                        yT_ps, v_sb[:, 2 * stp + k, :],
                                        e_sb[:, k * TC2:(k + 1) * TC2],
                                        start=(first and k == 0),
                                        stop=(last and k == 1))
                                for k in range(2):
                                    nc.tensor.matmul(
                                        cs_ps, ones,
                                        e_sb[:, k * TC2:(k + 1) * TC2],
                                        start=(first and k == 0),
                                        stop=(last and k == 1))

                            # software pipeline: scores(p+1) before y/cs(p)
                            prev = sc_pair(0)
                            for stp in range(1, TT_N // 2):
                                cur = sc_pair(stp)
                                yc_pair(stp - 1, prev)
                                prev = cur
                            yc_pair(TT_N // 2 - 1, prev)

                            cs_sb = smB.tile([1, TC2], fp32, tag="cssb")
                            nc.vector.tensor_copy(cs_sb, cs_ps)
                            rc_sb = smB.tile([1, TC2], fp32, tag="rcsb")
                            nc.vector.reciprocal(rc_sb, cs_sb)
                            rcslot = rc_dram[c2 * HG + h:c2 * HG + h + 1, :]
                            nc.sync.dma_start(out=rcslot, in_=rc_sb)
                            rb_sb = gB.tile([P, TC2], fp32, tag="rb")
                            nc.gpsimd.dma_start(
                                out=rb_sb, in_=rcslot.to_broadcast((P, TC2)))
                            g_sb = gB.tile([P, TC2], DT_IN, tag="g")
                            nc.sync.dma_start(
                                out=g_sb,
                                in_=gate_dram[h * P:(h + 1) * P, tsl])
                            gsc_sb = gB.tile([P, TC2], fp32, tag="gsc")
                            nc.vector.tensor_mul(gsc_sb, g_sb, rb_sb)
                            nc.vector.tensor_mul(ygT_sb[:, h, tsl], yT_ps,
                                                 gsc_sb)

                # ---------------- Phase C ----------------
                with tc.tile_pool(name="ost", bufs=3) as ostp, \
                     tc.tile_pool(name="psC", bufs=3, space="PSUM") as psC:
                    for tt in range(TT_N):
                        for et in range(C // 512):
                            o_ps = psC.tile([P, 512], fp32, tag="ops")
                            for hd in range(HG):
                                nc.tensor.matmul(
                                    o_ps,
                                    (ygT_sb[:, hd, tt * P:(tt + 1) * P]),
                                    (wproj_sb[:, hd,
                                                 et * 512:(et + 1) * 512]),
                                    start=(hd == 0), stop=(hd == HG - 1))
                            o_sb = ostp.tile([P, 512], fp32, tag="osb")
                            nc.vector.tensor_copy(o_sb, o_ps)
                            nc.sync.dma_start(
                                out=out_d[tt * P:(tt + 1) * P,
                                          et * 512:(et + 1) * 512],
                                in_=o_sb)
    nc.compile()
    return nc


def make_core_inputs(x, cos, sin, wq, wk, wv, w_gate, w_proj,
                     q_norm_w, k_norm_w, dt_mode=DT_MODE):
    """Host-side prep: per-core input dicts."""
    if dt_mode == "bf16":
        import ml_dtypes
        cdt = ml_dtypes.bfloat16
    else:
        cdt = np.float32

    cosf = np.asarray(cos, np.float32).reshape(T, 64)
    sinf = np.asarray(sin, np.float32).reshape(T, 64)
    qw = np.asarray(q_norm_w, np.float32)
    kw = np.asarray(k_norm_w, np.float32)
    ropeq = np.concatenate([cosf * qw[:64], sinf * qw[64:],
                            sinf * qw[:64], cosf * qw[64:]], axis=1)
    ropek = np.concatenate([cosf * kw[:64], sinf * kw[64:],
                            sinf * kw[:64], cosf * kw[64:]], axis=1)
    ropeq = np.ascontiguousarray(ropeq, np.float32)
    ropek = np.ascontiguousarray(ropek, np.float32)

    in_maps = []
    for core in range(N_CORES):
        b, g = core // NKV, core % NKV
        wqkv = np.concatenate([wq[:, g * GD:(g + 1) * GD],
                               wk[:, g * D:(g + 1) * D],
                               wv[:, g * D:(g + 1) * D]], axis=1)
        in_maps.append({
            "x": np.ascontiguousarray(x[b], np.float32).astype(cdt),
            "ident": np.eye(P, dtype=np.float32).astype(cdt),
            "wqkv": np.ascontiguousarray(wqkv, np.float32).astype(cdt),
            "wgate": np.ascontiguousarray(
                w_gate[:, g * GD:(g + 1) * GD], np.float32).astype(cdt),
            "wproj": np.ascontiguousarray(
                w_proj[g * GD:(g + 1) * GD, :], np.float32).astype(cdt),
            "ropeq": ropeq,
            "ropek": ropek,
        })
    return in_maps


def kernel(x, cos, sin, wq, wk, wv, w_gate, w_proj, q_norm_w, k_norm_w):
    from concourse.bass_utils import run_bass_kernel_spmd

    x = np.asarray(x)
    in_maps = make_core_inputs(x, cos, sin, wq, wk, wv, w_gate, w_proj,
                               q_norm_w, k_norm_w)
    nc = _build_nc(DT_MODE)
    res = run_bass_kernel_spmd(nc, in_maps, list(range(N_CORES)))
    partial = np.stack([res.results[i]["out"] for i in range(N_CORES)])
    out = partial.reshape(B, NKV, T, C).sum(axis=1)
    return out.astype(np.float32)
